# revision 37
# baseline (speedup 1.0000x reference)
"""Dual-stream fused attention kernel for 8 TRN2 NeuronCores.

Reference computation (B=2, N=2048, D=512, H=8, Dh=64):
    qkv_s = x_s @ W_qkv_s (s = 1,2)  -> per-head q_s, k_s, v_s
    dots  = SCALE * (q1 k1^T + q2 k2^T)          [b, h, n, n]
    attn  = softmax(dots)
    out_s = attn @ v_s                           [b, h, n, dh]
    out   = concat(merge(out1), merge(out2), axis=1) @ W_out + b_out

Sharding: core c handles batch b = c//4 and heads {2*(c%4), 2*(c%4)+1}
(data parallel on b, tensor parallel on h). Each core computes a partial
out-projection over its 128 inner columns; the host sums the 4 partials
per batch (the TP all-reduce) and adds b_out.

On-core dataflow (all matmuls bf16, fp32 PSUM accumulation):
  - QK projections produce transposed layouts QT/KT [d'=128, n] per head,
    with the two streams stacked on the contraction dim (d' = [s1 64 | s2 64]),
    so scores fuse the two streams in a single K=128 matmul.
  - Scores are computed transposed, S^T [k, q], so exp needs no transpose
    and P^T feeds the AV matmul directly as the moving operand.
  - Softmax is max-free (|SCALE * dots| <~ 1.5 for this problem's data
    distribution, exp cannot overflow); the denominator is accumulated on
    the vector engine (sum of P^T tiles over k-blocks) and reduced across
    partitions with a ones-vector matmul, then inverted with the fast
    approximate reciprocal (~18 correct bits, plenty for a softmax denom).
  - Normalization (1/rowsum) is fused into the PSUM evacuation of the AV
    accumulator (PSUM x SBUF -> SBUF multiply on the vector engine).

Engine assignment (the scalar/ACT engine paces the attention inner loop
with one exp per score tile, so everything else stays off it):
  - PE: all matmuls.
  - ACT: exp only.
  - DVE: denominator adds, reciprocal, fused normalize, projection-phase
    PSUM evacuations.
  - GPSIMD: attention-phase PSUM evacuations, out-projection staging.
  - SP/GPSIMD sequencers: all DMA dispatch (none on ACT/DVE).

The emission order software-pipelines the projections into the attention
loop: attention (qb0,h0) starts once kt[h0] block0 / qt[h0] / v quads for
the first key quarter exist; remaining projection tiles and the previous
q-block's out-projection are emitted as "hooks" between kb iterations.
"""

import numpy as np
import ml_dtypes

import bass_rust
import concourse.bass as bass
import concourse.mybir as mybir
import concourse.tile as tile
from concourse.bass_utils import run_bass_kernel_spmd

B, N, D = 2, 2048, 512
H, DH = 8, 64
SCALE = (2 * DH) ** -0.5
NCORES = 8
HPC = 2              # heads per core
CW = HPC * DH        # 128: per-core slice width of the inner dim
DC = D // 128        # 4 contraction chunks for the projections
NKB = N // 128       # 16 key blocks
QB = 1024            # q-block width for the attention inner loop
NQB = N // QB        # 2
BF16 = ml_dtypes.bfloat16


_WAIT_LIMIT = 1  # this container's walrus rejects multiple sync waits per instruction


def _split_sync_waits(nc):
    """Hoist excess semaphore waits onto same-engine NOPs inserted right
    before the over-budget instruction ("Too many sync wait commands")."""
    for f in nc.m.functions:
        for bb in f.blocks:
            insts = bb.instructions
            i = 0
            while i < len(insts):
                inst = insts[i]
                si = inst.sync_info
                if si is None:
                    i += 1
                    continue
                waits = list(si.on_wait)
                sem_waits = [w for w in waits if w.sync_type == "semaphore"]
                other = [w for w in waits if w.sync_type != "semaphore"]
                budget = _WAIT_LIMIT - len(other)
                if len(sem_waits) <= budget:
                    i += 1
                    continue
                keep = sem_waits[-budget:] if budget > 0 else []
                extra = sem_waits[:-budget] if budget > 0 else sem_waits
                for j in range(0, len(extra), _WAIT_LIMIT):
                    nop = mybir.InstNoOp(
                        name=f"I-{nc.next_id()}",
                        engine=inst.engine,
                        bass_nofuse=True,
                        sync_info=mybir.SyncInfo(
                            on_wait=extra[j:j + _WAIT_LIMIT], on_update=[]
                        ),
                    )
                    insts.insert(i, nop)
                    i += 1
                si.on_wait = other + keep
                inst.sync_info = si
                i += 1


def _body(nc, tc):
    bf = mybir.dt.bfloat16
    f32 = mybir.dt.float32
    EXP = mybir.ActivationFunctionType.Exp

    # All inputs arrive host-packed so every DMA descriptor is one long
    # contiguous DRAM run per partition (descriptor generation, ~5ns/row,
    # is the real dispatch cost): x comes as four fully-contiguous
    # 512-column chunks (128 descriptors of 4KB each), the output leaves
    # in a partition-major packed layout the host un-permutes.
    x1T = nc.dram_tensor("x1T", [4, 128, DC, 512], bf, kind="ExternalInput").ap()
    x2T = nc.dram_tensor("x2T", [4, 128, DC, 512], bf, kind="ExternalInput").ap()
    wq = [nc.dram_tensor(f"wq{s}", [128, DC * CW], bf, kind="ExternalInput").ap() for s in (1, 2)]
    wk = [nc.dram_tensor(f"wk{s}", [128, DC * CW], bf, kind="ExternalInput").ap() for s in (1, 2)]
    wv = [nc.dram_tensor(f"wv{s}", [128, DC * CW], bf, kind="ExternalInput").ap() for s in (1, 2)]
    wout = nc.dram_tensor("wout", [CW, D], bf, kind="ExternalInput").ap()
    out = nc.dram_tensor("out", [128, 2 * N // 128, D], bf, kind="ExternalOutput").ap()
    xT = [x1T, x2T]

    pools = []

    def mkpool(**kw):
        p = tc.alloc_tile_pool(**kw)
        pools.append(p)
        return p

    singles = mkpool(name="singles", bufs=1)
    # PSUM: exactly 8 banks total. spool holds [128,1024] score tiles and
    # doubles as the transient pool for projection / out-projection /
    # denominator tiles; avpool holds the double-buffered AV accumulators.
    spool = mkpool(name="spool", bufs=2, space="PSUM")    # 2x [128,1024] = 4 banks
    # avpool holds one slot per tag: "avA"/"avB" alternate as the AV
    # accumulator of successive (q-block, head) loops; the slot NOT currently
    # accumulating doubles as the transient psum for hook-emitted projection
    # and out-projection tiles (it is provably free from kb3 of the next loop
    # onward, once the deferred normalize has consumed it).
    avpool = mkpool(name="avpool", bufs=1, space="PSUM")  # 2x [128,1024] = 4 banks
    ptpool = mkpool(name="ptpool", bufs=6)
    accpool = mkpool(name="accpool", bufs=2)
    bcpool = mkpool(name="bcpool", bufs=2)

    # ---- input DMAs ------------------------------------------------------
    # Few LARGE dispatches spread across the four engine queues that are
    # idle at kernel start, ordered by first use: per stream, the first
    # column-quarter (needed by the projection start set) then the rest as
    # one dispatch. Weights are single-descriptor-per-partition loads.
    def load_w(ap, name, eng):
        t = singles.tile([128, DC, CW], bf, tag=name, name=name)
        eng.dma_start(out=t, in_=ap.rearrange("p (dc c) -> p dc c", dc=DC))
        return t

    x_all = [singles.tile([128, DC, N], bf, tag=f"x{s}", name=f"x{s}")
             for s in range(2)]
    x_sb = [[x_all[s][:, dc, :] for dc in range(DC)] for s in range(2)]

    def load_x_chunk(s, c, eng):
        eng.dma_start(out=x_all[s][:, :, c * 512:(c + 1) * 512], in_=xT[s][c])

    ones_mat = singles.tile([128, 128], bf, tag="ones", name="ones")
    nc.vector.memset(ones_mat, 1.0)
    act_scratch = singles.tile([1, 8], f32, tag="asc", name="asc")

    wq_sb = [None, None]
    wk_sb = [None, None]
    wv_sb = [None, None]
    # sync and scalar are the two hardware-DGE rings (~130 GB/s each);
    # gpsimd rides the (slower) software path. Weights go FIRST on the
    # rings (small, and the first projections block on them); the last x
    # chunks ride the software ring to spread transfer bandwidth.
    wk_sb[0] = load_w(wk[0], "wk1", nc.sync)
    wq_sb[0] = load_w(wq[0], "wq1", nc.scalar)
    load_x_chunk(0, 0, nc.sync)
    load_x_chunk(1, 0, nc.scalar)
    wv_sb[0] = load_w(wv[0], "wv1", nc.gpsimd)
    wv_sb[1] = load_w(wv[1], "wv2", nc.gpsimd)
    load_x_chunk(0, 1, nc.sync)
    load_x_chunk(1, 1, nc.scalar)
    # pull the ~2.7us exp table load onto the ACT queue NOW, so the first
    # real exp doesn't pay it mid-pipeline
    nc.scalar.activation(out=act_scratch, in_=ones_mat[0:1, 0:8], func=EXP)
    load_x_chunk(0, 2, nc.sync)
    load_x_chunk(1, 2, nc.scalar)
    wk_sb[1] = load_w(wk[1], "wk2", nc.gpsimd)
    wq_sb[1] = load_w(wq[1], "wq2", nc.gpsimd)
    load_x_chunk(0, 3, nc.gpsimd)
    load_x_chunk(1, 3, nc.gpsimd)
    wout_sb = singles.tile([CW, D], bf, tag="wout", name="wout")
    nc.gpsimd.dma_start(out=wout_sb, in_=wout)

    # ---- projection building blocks --------------------------------------
    qt = [singles.tile([128, N], bf, tag=f"qt{h}", name=f"qt{h}") for h in range(HPC)]
    kt = [singles.tile([128, N], bf, tag=f"kt{h}", name=f"kt{h}") for h in range(HPC)]
    v_all = singles.tile([128, NKB, HPC, 2, DH], bf, tag="vall", name="vall")
    w_for = {"q": wq_sb, "k": wk_sb}
    dst_for = {"q": qt, "k": kt}

    def _copy(eng, out, in_):
        if hasattr(eng, "tensor_copy"):
            eng.tensor_copy(out=out, in_=in_)
        else:
            eng.copy(out=out, in_=in_)

    def qk_tile(dst, h, nch, evac, pool, tag):
        """One [128, 512] column block of qt[h]/kt[h] (both streams)."""
        ps = pool.tile([128, 512], f32, tag=tag, name="qk")
        w_sb = w_for[dst]
        for s in range(2):
            for dc in range(DC):
                nc.tensor.matmul(
                    ps[s * 64:(s + 1) * 64, :],
                    lhsT=w_sb[s][:, dc, h * 64:(h + 1) * 64],
                    rhs=x_sb[s][dc][:, nch * 512:(nch + 1) * 512],
                    start=(dc == 0),
                    stop=(dc == DC - 1),
                )
        _copy(evac, dst_for[dst][h][:, nch * 512:(nch + 1) * 512], ps)

    def v_quad(s, qd, evac, pool, tag):
        """V proj for key quarter qd of stream s: 4 key blocks -> v_all."""
        ps = pool.tile([128, 512], f32, tag=tag, name="vq")
        for j in range(4):
            nb = qd * 4 + j
            for dc in range(DC):
                nc.tensor.matmul(
                    ps[:, j * 128:(j + 1) * 128],
                    lhsT=x_sb[s][dc][:, nb * 128:(nb + 1) * 128],
                    rhs=wv_sb[s][:, dc, :],
                    start=(dc == 0),
                    stop=(dc == DC - 1),
                )
        _copy(evac, v_all[:, qd * 4:(qd + 1) * 4, :, s, :],
              ps.rearrange("p (nb h d) -> p nb h d", nb=4, h=HPC))

    # ---- out-projection ---------------------------------------------------
    merged = [singles.tile([128, N], bf, tag=f"merged{s}", name=f"merged{s}")
              for s in range(2)]

    ostage = singles.tile([128, 8, 2, 512], bf, tag="ost", name="ost")

    def outproj_pair(qb, j, stage_eng, psum_pool, psum_tag, dma_eng):
        """Out-projection for two consecutive 128-row blocks of stream
        j//4: two matmuls into one psum slot, ONE staging cast over the
        whole pair, ONE output DMA."""
        s, rbj = divmod(j, 4)
        rb = qb * (QB // 128) + 2 * rbj
        ps = psum_pool.tile([128, QB], f32, tag=psum_tag, name="op")
        st = ostage[:, j, :, :]
        for half in range(2):
            nc.tensor.matmul(
                ps[:, half * 512:(half + 1) * 512],
                lhsT=merged[s][:, (rb + half) * 128:(rb + half + 1) * 128],
                rhs=wout_sb,
                start=True,
                stop=True,
            )
        _copy(stage_eng, st, ps.rearrange("p (rb d) -> p rb d", rb=2))
        g0 = s * (N // 128) + rb
        dma_eng.dma_start(out=out[:, g0:g0 + 2, :], in_=st)

    # ---- attention --------------------------------------------------------
    def attention(q0, qw, h, hooks, av_tag, o_eng=None, pre_av_hooks=None):
        """Emit one attention loop over q-columns [q0, q0+qw) of head h.
        The score matmul for kb+1 is emitted BEFORE the AV matmul of kb so
        the in-order PE queue never head-blocks on exp(kb); hook matmuls
        slot in between for the same reason.
        Returns "finish" hooks (denominator reduce + reciprocal + fused
        normalize) that the caller emits a few kb iterations into the NEXT
        loop, so the PE/DVE queues never head-block on the denominator
        chain at the loop boundary."""
        o_eng = o_eng or nc.gpsimd
        nqh = qw // 512
        av = avpool.tile([128, qw], f32, tag=av_tag, name="av")
        # three denominator partial-sum chains, none starting with a copy
        # (the first op of each chain adds two pt tiles directly):
        #   acc_e on DVE: kb 0,2,4,..,14   acc_d on DVE: kb 3,7,11,15
        #   acc_o on GPSIMD (SBUF-only op, legal there; a GPSIMD add is
        #   ~2.1us so it only gets the sparse every-4th-kb chain)
        acc_e = accpool.tile([128, qw], bf, tag="acce", name="acce")
        acc_o = accpool.tile([128, qw], bf, tag="acco", name="acco")
        acc_d = accpool.tile([128, qw], bf, tag="accd", name="accd")
        accs = (acc_e, acc_o, acc_d)
        sps_tiles = {}
        pt_tiles = {}

        def emit_scores(kb):
            sps = spool.tile([128, qw], f32, tag="s", name="s")
            for qh in range(nqh):
                nc.tensor.matmul(
                    sps[:, qh * 512:(qh + 1) * 512],
                    lhsT=kt[h][:, kb * 128:(kb + 1) * 128],
                    rhs=qt[h][:, q0 + qh * 512:q0 + (qh + 1) * 512],
                    start=True,
                    stop=True,
                )
            sps_tiles[kb] = sps

        emit_scores(0)
        for kb in range(NKB):
            pt = ptpool.tile([128, qw], bf, tag="pt", name="pt")
            nc.scalar.activation(out=pt, in_=sps_tiles.pop(kb), func=EXP,
                                 scale=SCALE)
            for hook in (pre_av_hooks or {}).get(kb, ()):
                hook()
            if kb + 1 < NKB:
                emit_scores(kb + 1)
            for qh in range(nqh):
                nc.tensor.matmul(
                    av[:, qh * 512:(qh + 1) * 512],
                    lhsT=v_all[:, kb, h, :, :],
                    rhs=pt[:, qh * 512:(qh + 1) * 512],
                    start=(kb == 0),
                    stop=(kb == NKB - 1),
                )
            pt_tiles[kb] = pt
            if kb % 2 == 0:
                acc, eng, first = acc_e, nc.vector, (kb == 2)
            elif kb % 4 == 1:
                acc, eng, first = acc_o, o_eng, (kb == 5)
            else:
                acc, eng, first = acc_d, nc.vector, (kb == 7)
            if (kb % 2 == 0 and kb >= 2) or kb >= 5:
                in0 = pt_tiles.pop(kb - (2 if kb % 2 == 0 else 4)) if first else acc
                eng.tensor_add(out=acc, in0=in0, in1=pt)
            for hook in hooks.get(kb, ()):
                hook()

        def fin_reduce():
            # ones[128,128].T @ acc replicates the column sums of acc into
            # every partition; one fast-approx reciprocal over the q-block.
            bc = spool.tile([128, qw], f32, tag="s", name="bc")
            for qh in range(nqh):
                for j, acc in enumerate(accs):
                    nc.tensor.matmul(
                        bc[:, qh * 512:(qh + 1) * 512],
                        lhsT=ones_mat,
                        rhs=acc[:, qh * 512:(qh + 1) * 512],
                        start=(j == 0),
                        stop=(j == len(accs) - 1),
                    )
            bcast = bcpool.tile([128, qw], f32, tag="bc", name="bc")
            nc.vector.reciprocal_approx_fast(out=bcast, in_=bc)
            fin_state.append(bcast)

        def fin_norm(c0=0, c1=1, n=1):
            # normalization fused into the AV PSUM evacuation; (c0, c1, n)
            # normalizes chunks [c0, c1) of n so the tail can interleave
            # out-projection per chunk
            cw = qw // n
            bcast = fin_state[-1]
            for ch in range(c0, c1):
                for s in range(2):
                    nc.vector.tensor_mul(
                        out=merged[s][h * 64:(h + 1) * 64,
                                      q0 + ch * cw:q0 + (ch + 1) * cw],
                        in0=av[s * 64:(s + 1) * 64, ch * cw:(ch + 1) * cw],
                        in1=bcast[s * 64:(s + 1) * 64, ch * cw:(ch + 1) * cw],
                    )
            if c1 == n:
                fin_state.pop()

        fin_state = []
        return [fin_reduce, fin_norm]

    # ---- emission ---------------------------------------------------------
    # HAM warm-up: throwaway accumulating matmuls bridging until the first
    # x chunks land (~13us: ~8.5us engine preamble before the first DMA
    # dispatch + transfer), so the PE is continuously busy from t=0, HAM
    # unthrottles at ~3.4us, and the projection start set runs at 2.4GHz.
    NWARM = 120
    warm = spool.tile([128, 512], f32, tag="s", name="warm")
    for i in range(NWARM):
        nc.tensor.matmul(warm[:, 0:128], lhsT=ones_mat, rhs=ones_mat,
                         start=(i == 0), stop=(i == NWARM - 1))

    # Start set: just enough projections for the first score matmul; the
    # first V quads ride as pre-AV hooks inside kb0 so they sit BEHIND the
    # first score/exp in the PE queue instead of ahead of it.
    qk_tile("k", 0, 0, nc.vector, spool, "s")
    qk_tile("q", 0, 0, nc.vector, spool, "s")
    qk_tile("q", 0, 1, nc.vector, spool, "s")

    gp = nc.gpsimd
    hook_evac = [nc.vector, nc.scalar]   # GPSIMD cannot read PSUM
    hook_n = [0]

    def qk_hook(dst, h, nch, tag):
        def f():
            hook_n[0] += 1
            qk_tile(dst, h, nch, hook_evac[hook_n[0] % 2], avpool, tag)
        return f

    def vq_hook(s, qd, tag):
        def f():
            hook_n[0] += 1
            v_quad(s, qd, hook_evac[hook_n[0] % 2], avpool, tag)
        return f

    hooks00 = {
        0: [qk_hook("k", 0, 1, "avB")],
        1: [vq_hook(0, 1, "avB")],
        2: [vq_hook(1, 1, "avB")],
        3: [qk_hook("k", 0, 2, "avB")],
        4: [qk_hook("k", 0, 3, "avB")],
        5: [vq_hook(0, 2, "avB")],
        6: [vq_hook(1, 2, "avB")],
        7: [vq_hook(0, 3, "avB")],
        8: [vq_hook(1, 3, "avB")],
        9: [qk_hook("q", 1, 0, "avB")],
        10: [qk_hook("q", 1, 1, "avB")],
        11: [qk_hook("k", 1, 0, "avB")],
        12: [qk_hook("k", 1, 1, "avB")],
    }
    pre00 = {0: [lambda: v_quad(0, 0, nc.vector, spool, "s"),
                 lambda: v_quad(1, 0, nc.scalar, spool, "s")]}
    fin = attention(0, QB, 0, hooks00, "avA", pre_av_hooks=pre00)
    hooks01 = {
        1: [fin[0]],
        3: [fin[1]],
        4: [qk_hook("k", 1, 2, "avA")],
        5: [qk_hook("k", 1, 3, "avA")],
        6: [qk_hook("q", 0, 2, "avA")],
        7: [qk_hook("q", 0, 3, "avA")],
    }
    fin = attention(0, QB, 1, hooks01, "avB")
    # qb0's out-projection rides inside (qb1, h0)/(qb1, h1); DMA dispatch
    # on the idle SP queue.
    def op0(j, tag):
        se = [nc.vector, nc.scalar][j % 2]
        de = [nc.sync, nc.scalar][j % 2]
        return lambda: outproj_pair(0, j, se, avpool, tag, de)

    hooks10 = {1: [fin[0]], 3: [fin[1]],
               5: [qk_hook("q", 1, 2, "avB")],
               7: [qk_hook("q", 1, 3, "avB")],
               4: [op0(0, "avB")], 9: [op0(1, "avB")], 12: [op0(2, "avB")]}
    fin = attention(QB, QB, 0, hooks10, "avA")
    # qb0's remaining out-projection pairs sit in the LATE kbs of the last
    # loop so the PE stays dense (and HAM stays warm) into the tail.
    hooks11 = {1: [fin[0]], 3: [fin[1]],
               4: [op0(3, "avA")], 6: [op0(4, "avA")], 9: [op0(5, "avA")],
               12: [op0(6, "avA")], 14: [op0(7, "avA")]}
    fin = attention(QB, QB, 1, hooks11, "avB")
    # tail: last head's denominator, then normalize 256-col chunks and emit
    # the matching out-projection pairs right behind each chunk, DMAing
    # straight from PSUM on rotating queues.
    fin[0]()
    tail_pools = [(spool, "s"), (avpool, "avA"), (spool, "s"), (avpool, "avA"),
                  (spool, "s"), (avpool, "avA"), (spool, "s"), (avpool, "avB")]
    tail_dma = [nc.sync, nc.scalar] * 4
    tail_stage = [nc.scalar, nc.vector]
    k = 0
    for ch in range(4):
        fin[1](ch, ch + 1, 4)
        for j in (ch, 4 + ch):
            pool, tag = tail_pools[k]
            outproj_pair(1, j, tail_stage[k % 2], pool, tag, tail_dma[k])
            k += 1

    for p in reversed(pools):
        p.release()


_NC_CACHE = None


def _build():
    global _NC_CACHE
    if _NC_CACHE is None:
        nc = bass.Bass("TRN2", target_bir_lowering=False, debug=False)
        with tile.TileContext(nc) as tc:
            _body(nc, tc)
        _split_sync_waits(nc)
        # populate .instr bytes for extended-inst InstISA subclasses (the
        # custom-DVE reciprocal) -- without this the NEFF compiler fails
        # with "ISA wrong length"
        from concourse.library_overlay import lower_extended_insts
        lower_extended_insts(nc)
        _NC_CACHE = nc
    return _NC_CACHE


def _pack_dc(a):
    """[D, cols] -> SBUF-tile layout [128, DC*cols] (partition-major, so
    each DMA descriptor is one contiguous >=1KB run per partition)."""
    d, cols = a.shape
    return np.ascontiguousarray(
        a.reshape(DC, 128, cols).transpose(1, 0, 2).reshape(128, DC * cols))


def _pack_x(a):
    """[N, D] activations -> [4, 128, DC, 512]: four fully-contiguous
    512-column chunks of the transposed [128, dc, n] SBUF layout."""
    return np.ascontiguousarray(
        a.T.reshape(DC, 128, 4, 512).transpose(2, 1, 0, 3))


def _prep_in_maps(x1, x2, W_qkv1, W_qkv2, W_out):
    x1 = np.asarray(x1, np.float32)
    x2 = np.asarray(x2, np.float32)
    W1 = np.asarray(W_qkv1, np.float32).astype(BF16)
    W2 = np.asarray(W_qkv2, np.float32).astype(BF16)
    Wo = np.asarray(W_out, np.float32).astype(BF16)
    xT = [
        [_pack_x(x[b].astype(BF16)) for b in range(B)]
        for x in (x1, x2)
    ]
    in_maps = []
    for c in range(NCORES):
        b, hg = divmod(c, NCORES // B)
        cs = slice(hg * CW, (hg + 1) * CW)
        in_maps.append({
            "x1T": xT[0][b],
            "x2T": xT[1][b],
            "wq1": _pack_dc(W1[:, 0:D][:, cs]),
            "wq2": _pack_dc(W2[:, 0:D][:, cs]),
            "wk1": _pack_dc(W1[:, D:2 * D][:, cs]),
            "wk2": _pack_dc(W2[:, D:2 * D][:, cs]),
            "wv1": _pack_dc(W1[:, 2 * D:3 * D][:, cs]),
            "wv2": _pack_dc(W2[:, 2 * D:3 * D][:, cs]),
            "wout": np.ascontiguousarray(Wo[cs, :]),
        })
    return in_maps


def _run(inputs, **spmd_kwargs):
    nc = _build()
    in_maps = _prep_in_maps(
        inputs["x1"], inputs["x2"], inputs["W_qkv1"], inputs["W_qkv2"],
        inputs["W_out"],
    )
    res = run_bass_kernel_spmd(nc, in_maps, core_ids=list(range(NCORES)),
                               **spmd_kwargs)
    b_out = np.asarray(inputs["b_out"], np.float32)
    gpc = NCORES // B
    full = np.zeros((B, 2 * N, D), np.float32)
    for c in range(NCORES):
        o = res.results[c]["out"].astype(np.float32)  # [128, 2N/128, D] packed
        full[c // gpc] += o.transpose(1, 0, 2).reshape(2 * N, D)
    full += b_out
    return full, res


def kernel(**inputs):
    full, _ = _run(inputs)
    return full



# revision 39
# speedup vs baseline: 1.0321x; 1.0321x over previous
"""Dual-stream fused attention kernel for 8 TRN2 NeuronCores.

Reference computation (B=2, N=2048, D=512, H=8, Dh=64):
    qkv_s = x_s @ W_qkv_s (s = 1,2)  -> per-head q_s, k_s, v_s
    dots  = SCALE * (q1 k1^T + q2 k2^T)          [b, h, n, n]
    attn  = softmax(dots)
    out_s = attn @ v_s                           [b, h, n, dh]
    out   = concat(merge(out1), merge(out2), axis=1) @ W_out + b_out

Sharding: core c handles batch b = c//4 and heads {2*(c%4), 2*(c%4)+1}
(data parallel on b, tensor parallel on h). Each core computes a partial
out-projection over its 128 inner columns; the host sums the 4 partials
per batch (the TP all-reduce) and adds b_out.

On-core dataflow (all matmuls bf16, fp32 PSUM accumulation):
  - QK projections produce transposed layouts QT/KT [d'=128, n] per head,
    with the two streams stacked on the contraction dim (d' = [s1 64 | s2 64]),
    so scores fuse the two streams in a single K=128 matmul.
  - Scores are computed transposed, S^T [k, q], so exp needs no transpose
    and P^T feeds the AV matmul directly as the moving operand.
  - Softmax is max-free (|SCALE * dots| <~ 1.5 for this problem's data
    distribution, exp cannot overflow); the denominator is accumulated on
    the vector engine (sum of P^T tiles over k-blocks) and reduced across
    partitions with a ones-vector matmul, then inverted with the fast
    approximate reciprocal (~18 correct bits, plenty for a softmax denom).
  - Normalization (1/rowsum) is fused into the PSUM evacuation of the AV
    accumulator (PSUM x SBUF -> SBUF multiply on the vector engine).

Engine assignment (the scalar/ACT engine paces the attention inner loop
with one exp per score tile, so everything else stays off it):
  - PE: all matmuls.
  - ACT: exp only.
  - DVE: denominator adds, reciprocal, fused normalize, projection-phase
    PSUM evacuations.
  - GPSIMD: attention-phase PSUM evacuations, out-projection staging.
  - SP/GPSIMD sequencers: all DMA dispatch (none on ACT/DVE).

The emission order software-pipelines the projections into the attention
loop: attention (qb0,h0) starts once kt[h0] block0 / qt[h0] / v quads for
the first key quarter exist; remaining projection tiles and the previous
q-block's out-projection are emitted as "hooks" between kb iterations.
"""

import numpy as np
import ml_dtypes

import bass_rust
import concourse.bass as bass
import concourse.mybir as mybir
import concourse.tile as tile
from concourse.bass_utils import run_bass_kernel_spmd

B, N, D = 2, 2048, 512
H, DH = 8, 64
SCALE = (2 * DH) ** -0.5
NCORES = 8
HPC = 2              # heads per core
CW = HPC * DH        # 128: per-core slice width of the inner dim
DC = D // 128        # 4 contraction chunks for the projections
NKB = N // 128       # 16 key blocks
QB = 1024            # q-block width for the attention inner loop
NQB = N // QB        # 2
BF16 = ml_dtypes.bfloat16


_WAIT_LIMIT = 1  # this container's walrus rejects multiple sync waits per instruction


def _split_sync_waits(nc):
    """Hoist excess semaphore waits onto same-engine NOPs inserted right
    before the over-budget instruction ("Too many sync wait commands")."""
    for f in nc.m.functions:
        for bb in f.blocks:
            insts = bb.instructions
            i = 0
            while i < len(insts):
                inst = insts[i]
                si = inst.sync_info
                if si is None:
                    i += 1
                    continue
                waits = list(si.on_wait)
                sem_waits = [w for w in waits if w.sync_type == "semaphore"]
                other = [w for w in waits if w.sync_type != "semaphore"]
                budget = _WAIT_LIMIT - len(other)
                if len(sem_waits) <= budget:
                    i += 1
                    continue
                keep = sem_waits[-budget:] if budget > 0 else []
                extra = sem_waits[:-budget] if budget > 0 else sem_waits
                for j in range(0, len(extra), _WAIT_LIMIT):
                    nop = mybir.InstNoOp(
                        name=f"I-{nc.next_id()}",
                        engine=inst.engine,
                        bass_nofuse=True,
                        sync_info=mybir.SyncInfo(
                            on_wait=extra[j:j + _WAIT_LIMIT], on_update=[]
                        ),
                    )
                    insts.insert(i, nop)
                    i += 1
                si.on_wait = other + keep
                inst.sync_info = si
                i += 1


def _body(nc, tc):
    bf = mybir.dt.bfloat16
    f32 = mybir.dt.float32
    EXP = mybir.ActivationFunctionType.Exp

    # All inputs arrive host-packed so every DMA descriptor is one long
    # contiguous DRAM run per partition (descriptor generation, ~5ns/row,
    # is the real dispatch cost): x comes as four fully-contiguous
    # 512-column chunks (128 descriptors of 4KB each), the output leaves
    # in a partition-major packed layout the host un-permutes.
    x1T = nc.dram_tensor("x1T", [4, 128, DC, 512], bf, kind="ExternalInput").ap()
    x2T = nc.dram_tensor("x2T", [4, 128, DC, 512], bf, kind="ExternalInput").ap()
    wq = [nc.dram_tensor(f"wq{s}", [128, DC * CW], bf, kind="ExternalInput").ap() for s in (1, 2)]
    wk = [nc.dram_tensor(f"wk{s}", [128, DC * CW], bf, kind="ExternalInput").ap() for s in (1, 2)]
    wv = [nc.dram_tensor(f"wv{s}", [128, DC * CW], bf, kind="ExternalInput").ap() for s in (1, 2)]
    wout = nc.dram_tensor("wout", [CW, D], bf, kind="ExternalInput").ap()
    out = nc.dram_tensor("out", [128, 2 * N // 128, D], bf, kind="ExternalOutput").ap()
    xT = [x1T, x2T]

    pools = []

    def mkpool(**kw):
        p = tc.alloc_tile_pool(**kw)
        pools.append(p)
        return p

    singles = mkpool(name="singles", bufs=1)
    # PSUM: exactly 8 banks total. spool holds [128,1024] score tiles and
    # doubles as the transient pool for projection / out-projection /
    # denominator tiles; avpool holds the double-buffered AV accumulators.
    spool = mkpool(name="spool", bufs=2, space="PSUM")    # 2x [128,1024] = 4 banks
    # avpool holds one slot per tag: "avA"/"avB" alternate as the AV
    # accumulator of successive (q-block, head) loops; the slot NOT currently
    # accumulating doubles as the transient psum for hook-emitted projection
    # and out-projection tiles (it is provably free from kb3 of the next loop
    # onward, once the deferred normalize has consumed it).
    avpool = mkpool(name="avpool", bufs=1, space="PSUM")  # 2x [128,1024] = 4 banks
    ptpool = mkpool(name="ptpool", bufs=6)
    accpool = mkpool(name="accpool", bufs=2)
    bcpool = mkpool(name="bcpool", bufs=2)

    # ---- input DMAs ------------------------------------------------------
    # Few LARGE dispatches spread across the four engine queues that are
    # idle at kernel start, ordered by first use: per stream, the first
    # column-quarter (needed by the projection start set) then the rest as
    # one dispatch. Weights are single-descriptor-per-partition loads.
    def load_w(ap, name, eng):
        t = singles.tile([128, DC, CW], bf, tag=name, name=name)
        eng.dma_start(out=t, in_=ap.rearrange("p (dc c) -> p dc c", dc=DC))
        return t

    x_all = [singles.tile([128, DC, N], bf, tag=f"x{s}", name=f"x{s}")
             for s in range(2)]
    x_sb = [[x_all[s][:, dc, :] for dc in range(DC)] for s in range(2)]

    def load_x_chunk(s, c, eng):
        eng.dma_start(out=x_all[s][:, :, c * 512:(c + 1) * 512], in_=xT[s][c])

    ones_mat = singles.tile([128, 128], bf, tag="ones", name="ones")
    nc.vector.memset(ones_mat, 1.0)
    act_scratch = singles.tile([1, 8], f32, tag="asc", name="asc")

    wq_sb = [None, None]
    wk_sb = [None, None]
    wv_sb = [None, None]
    # sync and scalar are the two hardware-DGE rings; gpsimd rides the
    # independent software path (~120 GB/s). x monopolizes the hardware
    # rings (its arrival gates the whole pipeline start); every weight
    # rides the software ring in need-order.
    wk_sb[0] = load_w(wk[0], "wk1", nc.gpsimd)
    wq_sb[0] = load_w(wq[0], "wq1", nc.gpsimd)
    load_x_chunk(0, 0, nc.sync)
    load_x_chunk(1, 0, nc.scalar)
    wv_sb[0] = load_w(wv[0], "wv1", nc.gpsimd)
    wv_sb[1] = load_w(wv[1], "wv2", nc.gpsimd)
    load_x_chunk(0, 1, nc.sync)
    load_x_chunk(1, 1, nc.scalar)
    # pull the ~2.7us exp table load onto the ACT queue NOW, so the first
    # real exp doesn't pay it mid-pipeline
    nc.scalar.activation(out=act_scratch, in_=ones_mat[0:1, 0:8], func=EXP)
    load_x_chunk(0, 2, nc.sync)
    load_x_chunk(1, 2, nc.scalar)
    wk_sb[1] = load_w(wk[1], "wk2", nc.gpsimd)
    wq_sb[1] = load_w(wq[1], "wq2", nc.gpsimd)
    load_x_chunk(0, 3, nc.sync)
    load_x_chunk(1, 3, nc.scalar)
    wout_sb = singles.tile([CW, D], bf, tag="wout", name="wout")
    nc.gpsimd.dma_start(out=wout_sb, in_=wout)

    # ---- projection building blocks --------------------------------------
    qt = [singles.tile([128, N], bf, tag=f"qt{h}", name=f"qt{h}") for h in range(HPC)]
    kt = [singles.tile([128, N], bf, tag=f"kt{h}", name=f"kt{h}") for h in range(HPC)]
    v_all = singles.tile([128, NKB, HPC, 2, DH], bf, tag="vall", name="vall")
    w_for = {"q": wq_sb, "k": wk_sb}
    dst_for = {"q": qt, "k": kt}

    def _copy(eng, out, in_):
        if hasattr(eng, "tensor_copy"):
            eng.tensor_copy(out=out, in_=in_)
        else:
            eng.copy(out=out, in_=in_)

    def qk_tile(dst, h, nch, evac, pool, tag):
        """One [128, 512] column block of qt[h]/kt[h] (both streams)."""
        ps = pool.tile([128, 512], f32, tag=tag, name="qk")
        w_sb = w_for[dst]
        for s in range(2):
            for dc in range(DC):
                nc.tensor.matmul(
                    ps[s * 64:(s + 1) * 64, :],
                    lhsT=w_sb[s][:, dc, h * 64:(h + 1) * 64],
                    rhs=x_sb[s][dc][:, nch * 512:(nch + 1) * 512],
                    start=(dc == 0),
                    stop=(dc == DC - 1),
                )
        _copy(evac, dst_for[dst][h][:, nch * 512:(nch + 1) * 512], ps)

    def v_quad(s, qd, evac, pool, tag):
        """V proj for key quarter qd of stream s: 4 key blocks -> v_all."""
        ps = pool.tile([128, 512], f32, tag=tag, name="vq")
        for j in range(4):
            nb = qd * 4 + j
            for dc in range(DC):
                nc.tensor.matmul(
                    ps[:, j * 128:(j + 1) * 128],
                    lhsT=x_sb[s][dc][:, nb * 128:(nb + 1) * 128],
                    rhs=wv_sb[s][:, dc, :],
                    start=(dc == 0),
                    stop=(dc == DC - 1),
                )
        _copy(evac, v_all[:, qd * 4:(qd + 1) * 4, :, s, :],
              ps.rearrange("p (nb h d) -> p nb h d", nb=4, h=HPC))

    # ---- out-projection ---------------------------------------------------
    merged = [singles.tile([128, N], bf, tag=f"merged{s}", name=f"merged{s}")
              for s in range(2)]

    ostage = singles.tile([128, 8, 2, 512], bf, tag="ost", name="ost")

    def outproj_pair(qb, j, stage_eng, psum_pool, psum_tag, dma_eng):
        """Out-projection for two consecutive 128-row blocks of stream
        j//4: two matmuls into one psum slot, ONE staging cast over the
        whole pair, ONE output DMA."""
        s, rbj = divmod(j, 4)
        rb = qb * (QB // 128) + 2 * rbj
        ps = psum_pool.tile([128, QB], f32, tag=psum_tag, name="op")
        st = ostage[:, j, :, :]
        for half in range(2):
            nc.tensor.matmul(
                ps[:, half * 512:(half + 1) * 512],
                lhsT=merged[s][:, (rb + half) * 128:(rb + half + 1) * 128],
                rhs=wout_sb,
                start=True,
                stop=True,
            )
        _copy(stage_eng, st, ps.rearrange("p (rb d) -> p rb d", rb=2))
        g0 = s * (N // 128) + rb
        dma_eng.dma_start(out=out[:, g0:g0 + 2, :], in_=st)

    # ---- attention --------------------------------------------------------
    def attention(q0, qw, h, hooks, av_tag, o_eng=None, pre_av_hooks=None):
        """Emit one attention loop over q-columns [q0, q0+qw) of head h.
        The score matmul for kb+1 is emitted BEFORE the AV matmul of kb so
        the in-order PE queue never head-blocks on exp(kb); hook matmuls
        slot in between for the same reason.
        Returns "finish" hooks (denominator reduce + reciprocal + fused
        normalize) that the caller emits a few kb iterations into the NEXT
        loop, so the PE/DVE queues never head-block on the denominator
        chain at the loop boundary."""
        o_eng = o_eng or nc.gpsimd
        nqh = qw // 512
        av = avpool.tile([128, qw], f32, tag=av_tag, name="av")
        # three denominator partial-sum chains, none starting with a copy
        # (the first op of each chain adds two pt tiles directly):
        #   acc_e on DVE: kb 0,2,4,..,14   acc_d on DVE: kb 3,7,11,15
        #   acc_o on GPSIMD (SBUF-only op, legal there; a GPSIMD add is
        #   ~2.1us so it only gets the sparse every-4th-kb chain)
        acc_e = accpool.tile([128, qw], bf, tag="acce", name="acce")
        acc_o = accpool.tile([128, qw], bf, tag="acco", name="acco")
        acc_d = accpool.tile([128, qw], bf, tag="accd", name="accd")
        accs = (acc_e, acc_o, acc_d)
        sps_tiles = {}
        pt_tiles = {}

        def emit_scores(kb):
            sps = spool.tile([128, qw], f32, tag="s", name="s")
            for qh in range(nqh):
                nc.tensor.matmul(
                    sps[:, qh * 512:(qh + 1) * 512],
                    lhsT=kt[h][:, kb * 128:(kb + 1) * 128],
                    rhs=qt[h][:, q0 + qh * 512:q0 + (qh + 1) * 512],
                    start=True,
                    stop=True,
                )
            sps_tiles[kb] = sps

        emit_scores(0)
        for kb in range(NKB):
            pt = ptpool.tile([128, qw], bf, tag="pt", name="pt")
            nc.scalar.activation(out=pt, in_=sps_tiles.pop(kb), func=EXP,
                                 scale=SCALE)
            for hook in (pre_av_hooks or {}).get(kb, ()):
                hook()
            if kb + 1 < NKB:
                emit_scores(kb + 1)
            for qh in range(nqh):
                nc.tensor.matmul(
                    av[:, qh * 512:(qh + 1) * 512],
                    lhsT=v_all[:, kb, h, :, :],
                    rhs=pt[:, qh * 512:(qh + 1) * 512],
                    start=(kb == 0),
                    stop=(kb == NKB - 1),
                )
            pt_tiles[kb] = pt
            if kb % 2 == 0:
                acc, eng, first = acc_e, nc.vector, (kb == 2)
            elif kb % 4 == 1:
                acc, eng, first = acc_o, o_eng, (kb == 5)
            else:
                acc, eng, first = acc_d, nc.vector, (kb == 7)
            if (kb % 2 == 0 and kb >= 2) or kb >= 5:
                in0 = pt_tiles.pop(kb - (2 if kb % 2 == 0 else 4)) if first else acc
                eng.tensor_add(out=acc, in0=in0, in1=pt)
            for hook in hooks.get(kb, ()):
                hook()

        def fin_reduce():
            # ones[128,128].T @ acc replicates the column sums of acc into
            # every partition; one fast-approx reciprocal over the q-block.
            bc = spool.tile([128, qw], f32, tag="s", name="bc")
            for qh in range(nqh):
                for j, acc in enumerate(accs):
                    nc.tensor.matmul(
                        bc[:, qh * 512:(qh + 1) * 512],
                        lhsT=ones_mat,
                        rhs=acc[:, qh * 512:(qh + 1) * 512],
                        start=(j == 0),
                        stop=(j == len(accs) - 1),
                    )
            bcast = bcpool.tile([128, qw], f32, tag="bc", name="bc")
            nc.vector.reciprocal_approx_fast(out=bcast, in_=bc)
            fin_state.append(bcast)

        def fin_norm(c0=0, c1=1, n=1):
            # normalization fused into the AV PSUM evacuation; (c0, c1, n)
            # normalizes chunks [c0, c1) of n so the tail can interleave
            # out-projection per chunk
            cw = qw // n
            bcast = fin_state[-1]
            for ch in range(c0, c1):
                for s in range(2):
                    nc.vector.tensor_mul(
                        out=merged[s][h * 64:(h + 1) * 64,
                                      q0 + ch * cw:q0 + (ch + 1) * cw],
                        in0=av[s * 64:(s + 1) * 64, ch * cw:(ch + 1) * cw],
                        in1=bcast[s * 64:(s + 1) * 64, ch * cw:(ch + 1) * cw],
                    )
            if c1 == n:
                fin_state.pop()

        fin_state = []
        return [fin_reduce, fin_norm]

    # ---- emission ---------------------------------------------------------
    # HAM warm-up: throwaway accumulating matmuls bridging until the first
    # x chunks land (~13us: ~8.5us engine preamble before the first DMA
    # dispatch + transfer), so the PE is continuously busy from t=0, HAM
    # unthrottles at ~3.4us, and the projection start set runs at 2.4GHz.
    NWARM = 120
    warm = spool.tile([128, 512], f32, tag="s", name="warm")
    for i in range(NWARM):
        nc.tensor.matmul(warm[:, 0:128], lhsT=ones_mat, rhs=ones_mat,
                         start=(i == 0), stop=(i == NWARM - 1))

    # Start set: just enough projections for the first score matmul; the
    # first V quads ride as pre-AV hooks inside kb0 so they sit BEHIND the
    # first score/exp in the PE queue instead of ahead of it.
    qk_tile("k", 0, 0, nc.vector, spool, "s")
    qk_tile("q", 0, 0, nc.vector, spool, "s")
    qk_tile("q", 0, 1, nc.vector, spool, "s")

    gp = nc.gpsimd
    hook_evac = [nc.vector, nc.scalar]   # GPSIMD cannot read PSUM
    hook_n = [0]

    def qk_hook(dst, h, nch, tag):
        def f():
            hook_n[0] += 1
            qk_tile(dst, h, nch, hook_evac[hook_n[0] % 2], avpool, tag)
        return f

    def vq_hook(s, qd, tag):
        def f():
            hook_n[0] += 1
            v_quad(s, qd, hook_evac[hook_n[0] % 2], avpool, tag)
        return f

    hooks00 = {
        0: [qk_hook("k", 0, 1, "avB")],
        1: [vq_hook(0, 1, "avB")],
        2: [vq_hook(1, 1, "avB")],
        3: [qk_hook("k", 0, 2, "avB")],
        4: [qk_hook("k", 0, 3, "avB")],
        5: [vq_hook(0, 2, "avB")],
        6: [vq_hook(1, 2, "avB")],
        7: [vq_hook(0, 3, "avB")],
        8: [vq_hook(1, 3, "avB")],
        9: [qk_hook("q", 1, 0, "avB")],
        10: [qk_hook("q", 1, 1, "avB")],
        11: [qk_hook("k", 1, 0, "avB")],
        12: [qk_hook("k", 1, 1, "avB")],
    }
    pre00 = {0: [lambda: v_quad(0, 0, nc.vector, spool, "s"),
                 lambda: v_quad(1, 0, nc.scalar, spool, "s")]}
    fin = attention(0, QB, 0, hooks00, "avA", pre_av_hooks=pre00)
    hooks01 = {
        1: [fin[0]],
        3: [fin[1]],
        4: [qk_hook("k", 1, 2, "avA")],
        5: [qk_hook("k", 1, 3, "avA")],
        6: [qk_hook("q", 0, 2, "avA")],
        7: [qk_hook("q", 0, 3, "avA")],
    }
    fin = attention(0, QB, 1, hooks01, "avB")
    # qb0's out-projection rides inside (qb1, h0)/(qb1, h1); DMA dispatch
    # on the idle SP queue.
    def op0(j, tag):
        se = [nc.vector, nc.scalar][j % 2]
        return lambda: outproj_pair(0, j, se, avpool, tag, nc.sync)

    hooks10 = {1: [fin[0]], 3: [fin[1]],
               5: [qk_hook("q", 1, 2, "avB")],
               7: [qk_hook("q", 1, 3, "avB")],
               4: [op0(0, "avB")], 9: [op0(1, "avB")], 12: [op0(2, "avB")]}
    fin = attention(QB, QB, 0, hooks10, "avA")
    # qb0's remaining out-projection pairs sit in the LATE kbs of the last
    # loop so the PE stays dense (and HAM stays warm) into the tail.
    hooks11 = {1: [fin[0]], 3: [fin[1]],
               4: [op0(3, "avA")], 6: [op0(4, "avA")], 9: [op0(5, "avA")],
               12: [op0(6, "avA")], 14: [op0(7, "avA")]}
    fin = attention(QB, QB, 1, hooks11, "avB")
    # tail: last head's denominator, then normalize 256-col chunks and emit
    # the matching out-projection pairs right behind each chunk, DMAing
    # straight from PSUM on rotating queues.
    fin[0]()
    tail_pools = [(spool, "s"), (avpool, "avA"), (spool, "s"), (avpool, "avA"),
                  (spool, "s"), (avpool, "avA"), (spool, "s"), (avpool, "avB")]
    tail_dma = [nc.sync, nc.scalar] * 4
    tail_stage = [nc.scalar, nc.vector]
    k = 0
    for ch in range(4):
        fin[1](ch, ch + 1, 4)
        for j in (ch, 4 + ch):
            pool, tag = tail_pools[k]
            outproj_pair(1, j, tail_stage[k % 2], pool, tag, tail_dma[k])
            k += 1

    for p in reversed(pools):
        p.release()


_NC_CACHE = None


def _build():
    global _NC_CACHE
    if _NC_CACHE is None:
        nc = bass.Bass("TRN2", target_bir_lowering=False, debug=False)
        with tile.TileContext(nc) as tc:
            _body(nc, tc)
        _split_sync_waits(nc)
        # populate .instr bytes for extended-inst InstISA subclasses (the
        # custom-DVE reciprocal) -- without this the NEFF compiler fails
        # with "ISA wrong length"
        from concourse.library_overlay import lower_extended_insts
        lower_extended_insts(nc)
        _NC_CACHE = nc
    return _NC_CACHE


def _pack_dc(a):
    """[D, cols] -> SBUF-tile layout [128, DC*cols] (partition-major, so
    each DMA descriptor is one contiguous >=1KB run per partition)."""
    d, cols = a.shape
    return np.ascontiguousarray(
        a.reshape(DC, 128, cols).transpose(1, 0, 2).reshape(128, DC * cols))


def _pack_x(a):
    """[N, D] activations -> [4, 128, DC, 512]: four fully-contiguous
    512-column chunks of the transposed [128, dc, n] SBUF layout."""
    return np.ascontiguousarray(
        a.T.reshape(DC, 128, 4, 512).transpose(2, 1, 0, 3))


def _prep_in_maps(x1, x2, W_qkv1, W_qkv2, W_out):
    x1 = np.asarray(x1, np.float32)
    x2 = np.asarray(x2, np.float32)
    W1 = np.asarray(W_qkv1, np.float32).astype(BF16)
    W2 = np.asarray(W_qkv2, np.float32).astype(BF16)
    Wo = np.asarray(W_out, np.float32).astype(BF16)
    xT = [
        [_pack_x(x[b].astype(BF16)) for b in range(B)]
        for x in (x1, x2)
    ]
    in_maps = []
    for c in range(NCORES):
        b, hg = divmod(c, NCORES // B)
        cs = slice(hg * CW, (hg + 1) * CW)
        in_maps.append({
            "x1T": xT[0][b],
            "x2T": xT[1][b],
            "wq1": _pack_dc(W1[:, 0:D][:, cs]),
            "wq2": _pack_dc(W2[:, 0:D][:, cs]),
            "wk1": _pack_dc(W1[:, D:2 * D][:, cs]),
            "wk2": _pack_dc(W2[:, D:2 * D][:, cs]),
            "wv1": _pack_dc(W1[:, 2 * D:3 * D][:, cs]),
            "wv2": _pack_dc(W2[:, 2 * D:3 * D][:, cs]),
            "wout": np.ascontiguousarray(Wo[cs, :]),
        })
    return in_maps


def _run(inputs, **spmd_kwargs):
    nc = _build()
    in_maps = _prep_in_maps(
        inputs["x1"], inputs["x2"], inputs["W_qkv1"], inputs["W_qkv2"],
        inputs["W_out"],
    )
    res = run_bass_kernel_spmd(nc, in_maps, core_ids=list(range(NCORES)),
                               **spmd_kwargs)
    b_out = np.asarray(inputs["b_out"], np.float32)
    gpc = NCORES // B
    full = np.zeros((B, 2 * N, D), np.float32)
    for c in range(NCORES):
        o = res.results[c]["out"].astype(np.float32)  # [128, 2N/128, D] packed
        full[c // gpc] += o.transpose(1, 0, 2).reshape(2 * N, D)
    full += b_out
    return full, res


def kernel(**inputs):
    full, _ = _run(inputs)
    return full

